# revision 59
# baseline (speedup 1.0000x reference)
"""JointFluxSingleTransformerBlockControl — TRN2 Bass kernel, 8-core tensor parallel.

Sharding (per core c of 8):
  - heads: 3 of 24  (q/k/v column-parallel, both streams)
  - mlp hidden: 1536 of 12288 rows
  - ada-norm emb rows: 1152 of 9216 (matvec sharded, device AllGather)
  - out-proj: column-parallel over this core's 1920 h-columns -> partial
    [3072, 1024] (T-layout) per stream; gate, out_b/8 and residual/8 are
    folded in on device so the host does a pure sum over cores.

Layout: activations in T-layout [feature=partition, seq=free]; weights are
pre-transposed on host so no on-device transposes are needed anywhere.
"""

import numpy as np

import concourse.bass as bass
import concourse.bacc as bacc
import concourse.tile as tile
from concourse import mybir
from concourse.bass_utils import run_bass_kernel_spmd

F32 = mybir.dt.float32
BF16 = mybir.dt.bfloat16
AF = mybir.ActivationFunctionType

D = 3072
S = 1024
HD = 128
NCORES = 8
HPC = 3                  # heads per core
QO = HPC * HD            # 384 q/k/v out-dims per core
MLPC = 12288 // NCORES   # 1536
ES = 9216 // NCORES      # 1152 e-rows per core
KC = D // 128            # 24 contraction chunks
EPS = 1e-6
INV_SQRT_HD = float(1.0 / np.sqrt(128.0))
HKC = (QO + MLPC) // 128  # 15 h-col chunks per core


def bcast(ap, p=128):
    """Partition-broadcast a free-dims-only AP to [p, *free]."""
    return bass.AP(tensor=ap.tensor, offset=ap.offset, ap=[[0, p]] + list(ap.ap))


# All per-core inputs are packed into two flat dram blobs (one per dtype) so
# each NEFF execution binds 3 input buffers instead of 31 (~30 us/input/exec
# of per-exec overhead measured through the pjrt path).
BF16_SPECS = [
    ("xT_m", 3072, 1024), ("xT_c", 3072, 1024),
    ("normT_m", 3072, 1152), ("normT_c", 3072, 1152),
    ("wqT_m", 3072, 384), ("wkT_m", 3072, 384), ("wvT_m", 3072, 384),
    ("wqT_c", 3072, 384), ("wkT_c", 3072, 384), ("wvT_c", 3072, 384),
    ("mlpT", 3072, 1536), ("outT", 1920, 3072),
    ("cosT", 128, 1024), ("sinT", 128, 1024), ("rotT", 128, 128),
]
F32_SPECS = [
    ("temb_m", 128, 24), ("temb_c", 128, 24),
    ("nb_m", 1, 1152), ("nb_c", 1, 1152),
    ("qb_m", 128, 3), ("kb_m", 128, 3), ("qb_c", 128, 3), ("kb_c", 128, 3),
    ("vb_m", 1, 384), ("vb_c", 1, 384),
    ("rmsq_m", 128, 1), ("rmsk_m", 128, 1),
    ("rmsq_c", 128, 1), ("rmsk_c", 128, 1),
    ("mlpb", 128, 12), ("outb", 128, 24),
]
OFF16, W16 = {}, {}
N16 = 0
for _n, _r, _c in BF16_SPECS:
    OFF16[_n], W16[_n] = N16, _c
    N16 += _r * _c
OFF32, W32 = {}, {}
N32 = 0
for _n, _r, _c in F32_SPECS:
    OFF32[_n], W32[_n] = N32, _c
    N32 += _r * _c


def build_nc():
    nc = bacc.Bacc(None, target_bir_lowering=False)
    dp = nc.declare_dram_parameter
    B16 = dp("b16", [1, N16], BF16, isOutput=False)
    B32 = dp("b32", [1, N32], F32, isOutput=False)
    b16, b32 = B16[:, :], B32[:, :]

    def A16(name, r0=0, nr=None, c0=0, ncol=None):
        W = W16[name]
        nr = 128 if nr is None else nr
        ncol = W if ncol is None else ncol
        return bass.AP(tensor=b16.tensor,
                       offset=b16.offset + OFF16[name] + r0 * W + c0,
                       ap=[[W, nr], [1, ncol]])

    def A32(name, r0=0, nr=None, c0=0, ncol=None):
        W = W32[name]
        nr = 128 if nr is None else nr
        ncol = W if ncol is None else ncol
        return bass.AP(tensor=b32.tensor,
                       offset=b32.offset + OFF32[name] + r0 * W + c0,
                       ap=[[W, nr], [1, ncol]])

    def A32_bcast(name, ncol):
        return bass.AP(tensor=b32.tensor, offset=b32.offset + OFF32[name],
                       ap=[[0, 128], [1, ncol]])

    OUTMC = dp("out_mc", [2 * D, S], F32, isOutput=True)
    OUT_OFF = {"m": 0, "c": D}

    with tile.TileContext(nc) as tc:
        with (
            tc.tile_pool(name="dram", bufs=1, space="DRAM") as dram,
            tc.tile_pool(name="const", bufs=1) as const,
            tc.tile_pool(name="psum", bufs=8, space="PSUM") as psum,
            tc.tile_pool(name="rows", bufs=1) as rows,
            tc.tile_pool(name="mlpres", bufs=1) as mres,
        ):
            ones = const.tile([128, 1], BF16)
            nc.vector.memset(ones, 1.0)
            epst = const.tile([128, 1], F32)
            nc.vector.memset(epst, EPS)
            rotT = const.tile([128, 128], BF16, tag="rotT")
            nc.sync.dma_start(rotT, A16("rotT"))
            cosT = const.tile([128, S], BF16, tag="cosT")
            sinT = const.tile([128, S], BF16, tag="sinT")
            nc.sync.dma_start(cosT, A16("cosT"))
            nc.sync.dma_start(sinT, A16("sinT"))
            mbt = const.tile([128, MLPC // 128], F32, tag="mlpb")
            nc.sync.dma_start(mbt, A32("mlpb"))
            qkb = {}
            rwt = {}
            vbb = {}
            for s in ("m", "c"):
                for pj in ("q", "k"):
                    t = const.tile([128, HPC], F32, tag=f"{pj}b_{s}")
                    nc.sync.dma_start(t, A32(f"{pj}b_{s}"))
                    qkb[(pj, s)] = t
                    r = const.tile([128, 1], F32, tag=f"rw_{pj}_{s}")
                    nc.sync.dma_start(r, A32(f"rms{pj}_{s}"))
                    rwt[(pj, s)] = r
                v = const.tile([128, QO], F32, tag=f"vb_{s}")
                nc.sync.dma_start(v, A32_bcast(f"vb_{s}", QO))
                vbb[s] = v

            # ---------------- phase E: ada-norm matvec + AllGather --------
            e_bounce = dram.tile([2, ES], F32)
            ag_out = dram.tile([2 * NCORES, ES], F32)
            with tc.tile_pool(name="ph_e", bufs=2) as pe:
                for si, s in enumerate(("m", "c")):
                    st_f = const.tile([128, KC], F32, tag=f"siluf_{s}")
                    nc.sync.dma_start(st_f, A32(f"temb_{s}"))
                    st_t = const.tile([128, KC], BF16, tag=f"silu_{s}")
                    nc.scalar.activation(st_t, st_f, AF.Silu)
                    eps_t = [psum.tile([1, 384], F32, tag="mm", name="eps_t") for _ in range(3)]
                    for kk in range(KC):
                        wn = pe.tile([128, ES], BF16, tag="wnorm")
                        nc.sync.dma_start(wn, A16(f"normT_{s}", kk * 128))
                        for nt in range(3):
                            nc.tensor.matmul(
                                eps_t[nt], st_t[:, kk:kk + 1],
                                wn[:, nt * 384:(nt + 1) * 384],
                                start=(kk == 0), stop=(kk == KC - 1))
                    erow = pe.tile([1, ES], F32, tag="erow")
                    nbr = pe.tile([1, ES], F32, tag="nbrow")
                    nc.sync.dma_start(nbr, A32(f"nb_{s}", 0, 1))
                    for nt in range(3):
                        sl = slice(nt * 384, (nt + 1) * 384)
                        nc.vector.tensor_add(erow[:, sl], eps_t[nt], nbr[:, sl])
                    nc.sync.dma_start(e_bounce[si:si + 1, :], erow)
            nc.gpsimd.collective_compute(
                "AllGather", mybir.AluOpType.bypass,
                replica_groups=[list(range(NCORES))],
                ins=[e_bounce.opt()], outs=[ag_out.opt()])
            # ag_out row (2c+si) = core c stream si. Flat e index j*128+p with
            # chunk j = 9c+jj (ES = 9*128): view [two, p, c, jj].
            ag4 = ag_out[:].rearrange("(c two) (jj p) -> two p c jj", two=2, p=128)
            ss, scale1, g_sb, ob8 = {}, {}, {}, {}
            outb_cc = const.tile([128, KC], F32, tag="outb_cc")
            nc.sync.dma_start(outb_cc, A32("outb"))
            for si, s in enumerate(("m", "c")):
                sst = const.tile([128, 48], F32, tag=f"ss_{s}")
                for cc in range(5):
                    nc.sync.dma_start(sst[:, cc * 9:(cc + 1) * 9],
                                      ag4[si, :, cc, :])
                nc.sync.dma_start(sst[:, 45:48], ag4[si, :, 5, 0:3])
                s1 = const.tile([128, KC], F32, tag=f"s1_{s}")
                nc.vector.tensor_scalar_add(s1, sst[:, 24:48], 1.0)
                ss[s], scale1[s] = sst, s1
                gt = const.tile([128, KC], F32, tag=f"gate_{s}")
                nc.sync.dma_start(gt[:, 0:6], ag4[si, :, 5, 3:9])
                for cc in (6, 7):
                    nc.sync.dma_start(gt[:, 6 + (cc - 6) * 9:6 + (cc - 5) * 9],
                                      ag4[si, :, cc, :])
                g_sb[s] = gt
                ot = const.tile([128, KC], F32, tag=f"ob8_{s}")
                nc.vector.tensor_mul(ot, gt, outb_cc)
                nc.vector.tensor_scalar_mul(ot, ot, 1.0 / NCORES)
                ob8[s] = ot

            rbounce = dram.tile([16, S], F32)
            rb_n = [0]

            def row_bcast(row_ap, dst_tile):
                i = rb_n[0] % 16
                rb_n[0] += 1
                nc.sync.dma_start(rbounce[i:i + 1, :], row_ap)
                nc.sync.dma_start(dst_tile, bcast(rbounce[i, :]))

            spq, spk, spv, spmlp = {}, {}, {}, {}
            with tc.tile_pool(name="nh", bufs=1) as nhp:
                for si, s in enumerate(("m", "c")):
                    # ---------- phase N: layernorm + ada scale/shift ------
                    nhT = nhp.tile([128, KC, S], BF16, tag="nhT")
                    with tc.tile_pool(name="ph_n", bufs=1) as pn:
                        sum_ps = [psum.tile([1, 512], F32, tag="mm", name="sum_ps") for _ in range(2)]
                        sq_ps = [psum.tile([1, 512], F32, tag="mm", name="sq_ps") for _ in range(2)]
                        xks = {}
                        for kk in range(KC):
                            xk = pn.tile([128, S], BF16, tag=f"xk{kk % 2}")
                            nc.sync.dma_start(xk, A16(f"xT_{s}", kk * 128))
                            sq = pn.tile([128, S], BF16, tag=f"xsq{kk % 2}")
                            nc.vector.tensor_mul(sq, xk, xk)
                            for st in range(2):
                                sl = slice(st * 512, (st + 1) * 512)
                                nc.tensor.matmul(sum_ps[st], ones, xk[:, sl],
                                                 start=(kk == 0), stop=(kk == KC - 1))
                                nc.tensor.matmul(sq_ps[st], ones, sq[:, sl],
                                                 start=(kk == 0), stop=(kk == KC - 1))
                        mu = pn.tile([1, S], F32, tag="mu")
                        msq = pn.tile([1, S], F32, tag="msq")
                        for st in range(2):
                            sl = slice(st * 512, (st + 1) * 512)
                            nc.scalar.activation(mu[:, sl], sum_ps[st], AF.Copy,
                                                 scale=1.0 / D)
                            nc.scalar.activation(msq[:, sl], sq_ps[st], AF.Copy,
                                                 scale=1.0 / D)
                        var = pn.tile([1, S], F32, tag="var")
                        nc.vector.tensor_mul(var, mu, mu)
                        nc.vector.tensor_sub(var, msq, var)
                        rstd = pn.tile([1, S], F32, tag="rstd")
                        nc.scalar.activation(rstd, var, AF.Sqrt, bias=epst[:1, :])
                        nc.vector.reciprocal(rstd, rstd)
                        nmr = pn.tile([1, S], F32, tag="nmr")
                        nc.vector.tensor_mul(nmr, mu, rstd)
                        nc.vector.tensor_scalar_mul(nmr, nmr, -1.0)
                        rstd_bc = pn.tile([128, S], F32, tag="rstd_bc")
                        nmr_bc = pn.tile([128, S], F32, tag="nmr_bc")
                        row_bcast(rstd, rstd_bc)
                        row_bcast(nmr, nmr_bc)
                        for kk in range(KC):
                            xk = pn.tile([128, S], BF16, tag=f"xk{kk % 2}")
                            nc.sync.dma_start(xk, A16(f"xT_{s}", kk * 128))
                            t1 = pn.tile([128, S], F32, tag=f"t1{kk % 2}")
                            nc.vector.tensor_mul(t1, xk, rstd_bc)
                            nc.vector.tensor_add(t1, t1, nmr_bc)
                            nc.scalar.activation(nhT[:, kk, :], t1, AF.Identity,
                                                 bias=ss[s][:, kk:kk + 1],
                                                 scale=scale1[s][:, kk:kk + 1])

                    # ---------- phase QKV ---------------------------------
                    with (
                        tc.tile_pool(name="ph_qkv1", bufs=1) as p1,
                        tc.tile_pool(name="ph_qkv2", bufs=2) as p2,
                        tc.tile_pool(name="ph_qkvw", bufs=3) as pw,
                    ):
                        for pj in ("q", "k"):
                            pps = [[psum.tile([128, 512], F32, tag="mm", name="pps")
                                    for _ in range(2)] for _ in range(HPC)]
                            for kk in range(KC):
                                wt = pw.tile([128, QO], BF16, tag="wqk")
                                nc.sync.dma_start(
                                    wt, A16(f"w{pj}T_{s}", kk * 128))
                                for o in range(HPC):
                                    for st in range(2):
                                        nc.tensor.matmul(
                                            pps[o][st], wt[:, o * 128:(o + 1) * 128],
                                            nhT[:, kk, st * 512:(st + 1) * 512],
                                            start=(kk == 0), stop=(kk == KC - 1))
                            spill = dram.tile([QO, S], BF16, tag=f"sp_{pj}_{s}")
                            (spq if pj == "q" else spk)[s] = spill
                            for o in range(HPC):
                                raw = p2.tile([128, S], F32, tag="raw")
                                for st in range(2):
                                    sl = slice(st * 512, (st + 1) * 512)
                                    nc.scalar.activation(
                                        raw[:, sl], pps[o][st], AF.Identity,
                                        bias=qkb[(pj, s)][:, o:o + 1])
                                sqh = p1.tile([128, S], BF16, tag="sqh")
                                nc.vector.tensor_mul(sqh, raw, raw)
                                rps = [psum.tile([1, 512], F32, tag="mm", name="rps")
                                       for _ in range(2)]
                                msr = p1.tile([1, S], F32, tag="msr")
                                for st in range(2):
                                    sl = slice(st * 512, (st + 1) * 512)
                                    nc.tensor.matmul(rps[st], ones, sqh[:, sl],
                                                     start=True, stop=True)
                                    nc.scalar.activation(msr[:, sl], rps[st],
                                                         AF.Copy, scale=1.0 / 128)
                                rsr = p1.tile([1, S], F32, tag="rsr")
                                nc.scalar.activation(rsr, msr, AF.Sqrt, bias=epst[:1, :])
                                nc.vector.reciprocal(rsr, rsr)
                                rs_bc = p1.tile([128, S], F32, tag="rs_bc")
                                row_bcast(rsr, rs_bc)
                                wq = p1.tile([128, S], BF16, tag="wq")
                                nc.scalar.activation(wq, raw, AF.Copy,
                                                     scale=rwt[(pj, s)])
                                rot_ps = [psum.tile([128, 512], F32, tag="mm", name="rot_ps")
                                          for _ in range(2)]
                                fin = p2.tile([128, S], F32, tag="fin")
                                for st in range(2):
                                    sl = slice(st * 512, (st + 1) * 512)
                                    nc.tensor.matmul(rot_ps[st], rotT, wq[:, sl],
                                                     start=True, stop=True)
                                    nc.vector.tensor_mul(fin[:, sl], rot_ps[st],
                                                         sinT[:, sl])
                                t2 = p1.tile([128, S], F32, tag="t2")
                                nc.vector.tensor_mul(t2, wq, cosT)
                                nc.vector.tensor_add(fin, fin, t2)
                                fin16 = p2.tile([128, S], BF16, tag="fin16")
                                nc.vector.tensor_mul(fin16, fin, rs_bc)
                                nc.sync.dma_start(spill[o * 128:(o + 1) * 128, :], fin16)

                        # v projection (natural layout [seq, 384])
                        vps = [psum.tile([128, QO], F32, tag="mm", name="vps") for _ in range(8)]
                        for kk in range(KC):
                            wt = pw.tile([128, QO], BF16, tag="wqk")
                            nc.sync.dma_start(
                                wt, A16(f"wvT_{s}", kk * 128))
                            for sc in range(8):
                                nc.tensor.matmul(
                                    vps[sc], nhT[:, kk, sc * 128:(sc + 1) * 128],
                                    wt, start=(kk == 0), stop=(kk == KC - 1))
                        vsp = dram.tile([S, QO], BF16, tag=f"sp_v_{s}")
                        spv[s] = vsp
                        for sc in range(8):
                            vt = p1.tile([128, QO], BF16, tag="vt")
                            nc.vector.tensor_add(vt, vps[sc], vbb[s])
                            nc.sync.dma_start(vsp[sc * 128:(sc + 1) * 128, :], vt)

                    # ---------- phase MLP ---------------------------------
                    # MLP activations stay resident in SBUF through OUT-PROJ
                    # (24 KB/partition per stream) — no DRAM spill round-trip.
                    msp = mres.tile([128, MLPC // 128, S], BF16, tag=f"hmlp_{s}")
                    spmlp[s] = msp
                    with (
                        tc.tile_pool(name="ph_mlpw", bufs=3) as mw,
                    ):
                        for ob in range(3):
                            mps = [[psum.tile([128, 512], F32, tag="mm", name="mps")
                                    for _ in range(2)] for _ in range(4)]
                            for kk in range(KC):
                                wt = mw.tile([128, 512], BF16, tag="wmlp")
                                nc.sync.dma_start(
                                    wt, A16("mlpT", kk * 128, 128, ob * 512, 512))
                                for o4 in range(4):
                                    for st in range(2):
                                        nc.tensor.matmul(
                                            mps[o4][st],
                                            wt[:, o4 * 128:(o4 + 1) * 128],
                                            nhT[:, kk, st * 512:(st + 1) * 512],
                                            start=(kk == 0), stop=(kk == KC - 1))
                            for o4 in range(4):
                                o = ob * 4 + o4
                                for st in range(2):
                                    sl = slice(st * 512, (st + 1) * 512)
                                    nc.scalar.activation(msp[:, o, sl], mps[o4][st],
                                                         AF.Gelu_apprx_tanh,
                                                         bias=mbt[:, o:o + 1])

            # ---------------- phase ATTN ----------------------------------
            with tc.tile_pool(name="attn_out", bufs=1) as ao:
              with (
                tc.tile_pool(name="attn_qkv", bufs=1) as aq,
                tc.tile_pool(name="attn_wk", bufs=3) as awk,
                tc.tile_pool(name="attn_w1", bufs=2) as aw1,
              ):
                qm = aq.tile([128, HPC, S], BF16, tag="qm")
                am = ao.tile([128, HPC, S], BF16, tag="am")
                ac = ao.tile([128, HPC, S], BF16, tag="ac")
                att_out = {"m": am, "c": ac}
                for h in range(HPC):
                    nc.sync.dma_start(qm[:, h, :], spq["m"][h * 128:(h + 1) * 128, :])
                kt = vt = None
                for attn, (qs, ks, acc_t, fresh) in (
                    ("main", ("m", "m", am, True)),
                    ("ctrl", ("c", "c", ac, True)),
                    ("cross", ("m", "c", am, False)),
                ):
                    if attn != "cross":
                        kt = aq.tile([128, HPC, S], BF16, tag="kt")
                        vt = aq.tile([128, 8, QO], BF16, tag="vt")
                        for h in range(HPC):
                            nc.sync.dma_start(kt[:, h, :],
                                              spk[ks][h * 128:(h + 1) * 128, :])
                        for sc in range(8):
                            nc.sync.dma_start(vt[:, sc, :],
                                              spv[ks][sc * 128:(sc + 1) * 128, :])
                    if attn == "ctrl":
                        qt = aq.tile([128, HPC, S], BF16, tag="qc")
                        for h in range(HPC):
                            nc.sync.dma_start(qt[:, h, :],
                                              spq["c"][h * 128:(h + 1) * 128, :])
                    else:
                        qt = qm
                    for h in range(HPC):
                        av_ps = [psum.tile([128, 512], F32, tag="mm", name="av_ps") for _ in range(2)]
                        d_ps = [psum.tile([1, 512], F32, tag="mm", name="d_ps") for _ in range(2)]
                        for kv in range(8):
                            s_ps = [psum.tile([128, 512], F32, tag="mm", name="s_ps")
                                    for _ in range(2)]
                            et = awk.tile([128, S], BF16, tag="exp")
                            for st in range(2):
                                sl = slice(st * 512, (st + 1) * 512)
                                nc.tensor.matmul(s_ps[st],
                                                 kt[:, h, kv * 128:(kv + 1) * 128],
                                                 qt[:, h, sl], start=True, stop=True)
                                nc.scalar.activation(et[:, sl], s_ps[st], AF.Exp,
                                                     scale=INV_SQRT_HD)
                                nc.tensor.matmul(av_ps[st],
                                                 vt[:, kv, h * 128:(h + 1) * 128],
                                                 et[:, sl], start=(kv == 0),
                                                 stop=(kv == 7))
                                nc.tensor.matmul(d_ps[st], ones, et[:, sl],
                                                 start=(kv == 0), stop=(kv == 7))
                        den = aw1.tile([1, S], F32, tag="den")
                        for st in range(2):
                            nc.vector.reciprocal(den[:, st * 512:(st + 1) * 512],
                                                 d_ps[st])
                        den_bc = aw1.tile([128, S], F32, tag="den_bc")
                        row_bcast(den, den_bc)
                        for st in range(2):
                            sl = slice(st * 512, (st + 1) * 512)
                            if fresh:
                                nc.vector.tensor_mul(acc_t[:, h, sl], av_ps[st],
                                                     den_bc[:, sl])
                            else:
                                cr = aw1.tile([128, 512], F32, tag="crs")
                                nc.vector.tensor_mul(cr, av_ps[st], den_bc[:, sl])
                                nc.vector.tensor_add(acc_t[:, h, sl],
                                                     acc_t[:, h, sl], cr)

              # ---------------- phase OUT-PROJ --------------------------
              with (
                  tc.tile_pool(name="ow", bufs=2) as ow,
                  tc.tile_pool(name="fin", bufs=4) as fp,
              ):
                  mlp_sb = spmlp  # resident SBUF arenas from phase MLP
                  WOUT = W16["outT"]
                  for oc in range(KC):
                      warena = ow.tile([128, HKC, 128], BF16, tag="warena")
                      # one strided DMA for all 15 [128,128] weight chunks
                      nc.sync.dma_start(
                          warena,
                          bass.AP(tensor=b16.tensor,
                                  offset=b16.offset + OFF16["outT"] + oc * 128,
                                  ap=[[WOUT, 128], [128 * WOUT, HKC], [1, 128]]))
                      for s in ("m", "c"):
                          ops_t = [psum.tile([128, 512], F32, tag="mm", name="ops_t")
                                   for _ in range(2)]
                          for kk in range(HKC):
                              rh = (att_out[s][:, kk, :] if kk < HPC
                                    else mlp_sb[s][:, kk - HPC, :])
                              for st in range(2):
                                  nc.tensor.matmul(
                                      ops_t[st], warena[:, kk, :],
                                      rh[:, st * 512:(st + 1) * 512],
                                      start=(kk == 0), stop=(kk == HKC - 1))
                          tg = fp.tile([128, S], F32, tag="tg")
                          xs = fp.tile([128, S], BF16, tag="xs")
                          nc.sync.dma_start(xs, A16(f"xT_{s}", oc * 128, 128))
                          xsf = fp.tile([128, S], F32, tag="xsf")
                          nc.vector.tensor_scalar_mul(xsf, xs, 1.0 / NCORES)
                          for st in range(2):
                              sl = slice(st * 512, (st + 1) * 512)
                              nc.scalar.activation(tg[:, sl], ops_t[st],
                                                   AF.Identity,
                                                   scale=g_sb[s][:, oc:oc + 1],
                                                   bias=ob8[s][:, oc:oc + 1])
                              nc.vector.tensor_add(tg[:, sl], tg[:, sl],
                                                   xsf[:, sl])
                          nc.sync.dma_start(
                              OUTMC[OUT_OFF[s] + oc * 128:
                                    OUT_OFF[s] + (oc + 1) * 128, :], tg)
    nc.compile()
    return nc


_NC_CACHE = []


def _get_nc():
    if not _NC_CACHE:
        _NC_CACHE.append(build_nc())
    return _NC_CACHE[0]


def _prep_core_inputs(inputs, c):
    f = np.float32
    bf = mybir.dt.np(BF16)
    T = lambda a: np.ascontiguousarray(np.asarray(a, f).T)
    TB = lambda a: np.ascontiguousarray(np.asarray(a, f).T.astype(bf))
    C = lambda a: np.ascontiguousarray(np.asarray(a, f))
    hs = np.asarray(inputs["hidden_states"], f)[0]
    hc = np.asarray(inputs["hidden_states_control"], f)[0]
    m = {}
    for s, x, te, nw, nb in (
        ("m", hs, inputs["temb"], inputs["norm_w"], inputs["norm_b"]),
        ("c", hc, inputs["temb_control"], inputs["normc_w"], inputs["normc_b"]),
    ):
        m[f"xT_{s}"] = TB(x)
        m[f"temb_{s}"] = C(np.asarray(te, f)[0].reshape(KC, 128).T)
        m[f"normT_{s}"] = TB(np.asarray(nw, f)[c * ES:(c + 1) * ES, :])
        m[f"nb_{s}"] = C(np.asarray(nb, f)[c * ES:(c + 1) * ES].reshape(1, ES))
    for s, pre in (("m", ""), ("c", "c")):
        for w in ("q", "k", "v"):
            W = np.asarray(inputs[f"{w}{pre}_w"], f)
            m[f"w{w}T_{s}"] = TB(W[c * QO:(c + 1) * QO, :])
        for w in ("q", "k"):
            b = np.asarray(inputs[f"{w}{pre}_b"], f)[c * QO:(c + 1) * QO]
            m[f"{w}b_{s}"] = C(b.reshape(HPC, 128).T)
        m[f"vb_{s}"] = C(np.asarray(inputs[f"v{pre}_b"], f)[c * QO:(c + 1) * QO]
                         .reshape(1, QO))
        m[f"rmsq_{s}"] = C(np.asarray(inputs["rms_q" + pre], f).reshape(128, 1))
        m[f"rmsk_{s}"] = C(np.asarray(inputs["rms_k" + pre], f).reshape(128, 1))
    m["mlpT"] = TB(np.asarray(inputs["mlp_w"], f)[c * MLPC:(c + 1) * MLPC, :])
    m["mlpb"] = C(np.asarray(inputs["mlp_b"], f)[c * MLPC:(c + 1) * MLPC]
                  .reshape(MLPC // 128, 128).T)
    ow = np.asarray(inputs["out_w"], f)
    m["outT"] = C(np.concatenate(
        [ow[:, c * QO:(c + 1) * QO], ow[:, D + c * MLPC:D + (c + 1) * MLPC]],
        axis=1).T).astype(bf)
    m["outb"] = C(np.asarray(inputs["out_b"], f).reshape(KC, 128).T)
    m["cosT"] = TB(inputs["rope_cos"])
    m["sinT"] = TB(inputs["rope_sin"])
    R = np.zeros((128, 128), f)
    for i in range(64):
        R[2 * i, 2 * i + 1] = -1.0
        R[2 * i + 1, 2 * i] = 1.0
    m["rotT"] = np.ascontiguousarray(R.T).astype(bf)
    for n, r, cc in BF16_SPECS:
        assert m[n].shape == (r, cc), (n, m[n].shape, (r, cc))
    for n, r, cc in F32_SPECS:
        assert m[n].shape == (r, cc), (n, m[n].shape, (r, cc))
    b16 = np.concatenate([np.asarray(m[n], bf).reshape(-1) for n, _, _ in BF16_SPECS])
    b32 = np.concatenate([np.asarray(m[n], f).reshape(-1) for n, _, _ in F32_SPECS])
    return {"b16": b16.reshape(1, N16), "b32": b32.reshape(1, N32)}


def run_cores(inputs, trace=False):
    nc = _get_nc()
    in_maps = [_prep_core_inputs(inputs, c) for c in range(NCORES)]
    res = run_bass_kernel_spmd(nc, in_maps, list(range(NCORES)), trace=trace)
    h = np.sum([r["out_mc"][:D] for r in res.results], axis=0, dtype=np.float64)
    hc = np.sum([r["out_mc"][D:] for r in res.results], axis=0, dtype=np.float64)
    h = np.ascontiguousarray(h.T.astype(np.float32)).reshape(1, S, D)
    hc = np.ascontiguousarray(hc.T.astype(np.float32)).reshape(1, S, D)
    return (h, hc), res


def kernel(**inputs):
    out, _ = run_cores(inputs, trace=False)
    return out



# revision 64
# speedup vs baseline: 1.2164x; 1.2164x over previous
"""JointFluxSingleTransformerBlockControl — TRN2 Bass kernel, 8-core tensor parallel.

Sharding (per core c of 8):
  - heads: 3 of 24  (q/k/v column-parallel, both streams)
  - mlp hidden: 1536 of 12288 rows
  - ada-norm emb rows: 1152 of 9216 (matvec sharded, device AllGather)
  - out-proj: column-parallel over this core's 1920 h-columns -> partial
    [3072, 1024] (T-layout) per stream; gate, out_b/8 and residual/8 are
    folded in on device so the host does a pure sum over cores.

Layout: activations in T-layout [feature=partition, seq=free]; weights are
pre-transposed on host so no on-device transposes are needed anywhere.
"""

import numpy as np

import concourse.bass as bass
import concourse.bacc as bacc
import concourse.tile as tile
from concourse import mybir
from concourse.bass_utils import run_bass_kernel_spmd

F32 = mybir.dt.float32
BF16 = mybir.dt.bfloat16
AF = mybir.ActivationFunctionType

D = 3072
S = 1024
HD = 128
NCORES = 8
HPC = 3                  # heads per core
QO = HPC * HD            # 384 q/k/v out-dims per core
MLPC = 12288 // NCORES   # 1536
ES = 9216 // NCORES      # 1152 e-rows per core
KC = D // 128            # 24 contraction chunks
EPS = 1e-6
INV_SQRT_HD = float(1.0 / np.sqrt(128.0))
HKC = (QO + MLPC) // 128  # 15 h-col chunks per core


def bcast(ap, p=128):
    """Partition-broadcast a free-dims-only AP to [p, *free]."""
    return bass.AP(tensor=ap.tensor, offset=ap.offset, ap=[[0, p]] + list(ap.ap))


# All per-core inputs are packed into two flat dram blobs (one per dtype) so
# each NEFF execution binds 3 input buffers instead of 31 (~30 us/input/exec
# of per-exec overhead measured through the pjrt path).
BF16_SPECS = [
    ("xT_m", 3072, 1024), ("xT_c", 3072, 1024),
    ("normT_m", 3072, 1152), ("normT_c", 3072, 1152),
    ("wqT_m", 3072, 384), ("wkT_m", 3072, 384), ("wvT_m", 3072, 384),
    ("wqT_c", 3072, 384), ("wkT_c", 3072, 384), ("wvT_c", 3072, 384),
    ("mlpT", 3072, 1536), ("outT", 1920, 3072),
    ("cosT", 128, 1024), ("sinT", 128, 1024), ("rotT", 128, 128),
]
F32_SPECS = [
    ("temb_m", 128, 24), ("temb_c", 128, 24),
    ("nb_m", 1, 1152), ("nb_c", 1, 1152),
    ("qb_m", 128, 3), ("kb_m", 128, 3), ("qb_c", 128, 3), ("kb_c", 128, 3),
    ("vb_m", 1, 384), ("vb_c", 1, 384),
    ("rmsq_m", 128, 1), ("rmsk_m", 128, 1),
    ("rmsq_c", 128, 1), ("rmsk_c", 128, 1),
    ("mlpb", 128, 12), ("outb", 128, 24),
]
OFF16, W16 = {}, {}
N16 = 0
for _n, _r, _c in BF16_SPECS:
    OFF16[_n], W16[_n] = N16, _c
    N16 += _r * _c
OFF32, W32 = {}, {}
N32 = 0
for _n, _r, _c in F32_SPECS:
    OFF32[_n], W32[_n] = N32, _c
    N32 += _r * _c


def build_nc():
    nc = bacc.Bacc(None, target_bir_lowering=False)
    dp = nc.declare_dram_parameter
    B16 = dp("b16", [1, N16], BF16, isOutput=False)
    B32 = dp("b32", [1, N32], F32, isOutput=False)
    b16, b32 = B16[:, :], B32[:, :]

    def A16(name, r0=0, nr=None, c0=0, ncol=None):
        W = W16[name]
        nr = 128 if nr is None else nr
        ncol = W if ncol is None else ncol
        return bass.AP(tensor=b16.tensor,
                       offset=b16.offset + OFF16[name] + r0 * W + c0,
                       ap=[[W, nr], [1, ncol]])

    def A32(name, r0=0, nr=None, c0=0, ncol=None):
        W = W32[name]
        nr = 128 if nr is None else nr
        ncol = W if ncol is None else ncol
        return bass.AP(tensor=b32.tensor,
                       offset=b32.offset + OFF32[name] + r0 * W + c0,
                       ap=[[W, nr], [1, ncol]])

    def A32_bcast(name, ncol):
        return bass.AP(tensor=b32.tensor, offset=b32.offset + OFF32[name],
                       ap=[[0, 128], [1, ncol]])

    OUTMC = dp("out_mc", [2 * D, S], BF16, isOutput=True)
    OUT_OFF = {"m": 0, "c": D}

    with tile.TileContext(nc) as tc:
        with (
            tc.tile_pool(name="dram", bufs=1, space="DRAM") as dram,
            tc.tile_pool(name="const", bufs=1) as const,
            tc.tile_pool(name="psum", bufs=8, space="PSUM") as psum,
            tc.tile_pool(name="rows", bufs=1) as rows,
            tc.tile_pool(name="mlpres", bufs=1) as mres,
        ):
            ones = const.tile([128, 1], BF16)
            nc.vector.memset(ones, 1.0)
            epst = const.tile([128, 1], F32)
            nc.vector.memset(epst, EPS)
            rotT = const.tile([128, 128], BF16, tag="rotT")
            nc.sync.dma_start(rotT, A16("rotT"))
            cosT = const.tile([128, S], BF16, tag="cosT")
            sinT = const.tile([128, S], BF16, tag="sinT")
            nc.sync.dma_start(cosT, A16("cosT"))
            nc.sync.dma_start(sinT, A16("sinT"))
            mbt = const.tile([128, MLPC // 128], F32, tag="mlpb")
            nc.sync.dma_start(mbt, A32("mlpb"))
            qkb = {}
            rwt = {}
            vbb = {}
            for s in ("m", "c"):
                for pj in ("q", "k"):
                    t = const.tile([128, HPC], F32, tag=f"{pj}b_{s}")
                    nc.sync.dma_start(t, A32(f"{pj}b_{s}"))
                    qkb[(pj, s)] = t
                    r = const.tile([128, 1], F32, tag=f"rw_{pj}_{s}")
                    nc.sync.dma_start(r, A32(f"rms{pj}_{s}"))
                    rwt[(pj, s)] = r
                v = const.tile([128, QO], F32, tag=f"vb_{s}")
                nc.sync.dma_start(v, A32_bcast(f"vb_{s}", QO))
                vbb[s] = v

            # ---------------- phase E: ada-norm matvec + AllGather --------
            e_bounce = dram.tile([2, ES], F32)
            ag_out = dram.tile([2 * NCORES, ES], F32)
            with tc.tile_pool(name="ph_e", bufs=2) as pe:
                for si, s in enumerate(("m", "c")):
                    st_f = const.tile([128, KC], F32, tag=f"siluf_{s}")
                    nc.sync.dma_start(st_f, A32(f"temb_{s}"))
                    st_t = const.tile([128, KC], BF16, tag=f"silu_{s}")
                    nc.scalar.activation(st_t, st_f, AF.Silu)
                    eps_t = [psum.tile([1, 384], F32, tag="mm", name="eps_t") for _ in range(3)]
                    for kk in range(KC):
                        wn = pe.tile([128, ES], BF16, tag="wnorm")
                        nc.sync.dma_start(wn, A16(f"normT_{s}", kk * 128))
                        for nt in range(3):
                            nc.tensor.matmul(
                                eps_t[nt], st_t[:, kk:kk + 1],
                                wn[:, nt * 384:(nt + 1) * 384],
                                start=(kk == 0), stop=(kk == KC - 1))
                    erow = pe.tile([1, ES], F32, tag="erow")
                    nbr = pe.tile([1, ES], F32, tag="nbrow")
                    nc.sync.dma_start(nbr, A32(f"nb_{s}", 0, 1))
                    for nt in range(3):
                        sl = slice(nt * 384, (nt + 1) * 384)
                        nc.vector.tensor_add(erow[:, sl], eps_t[nt], nbr[:, sl])
                    nc.sync.dma_start(e_bounce[si:si + 1, :], erow)
            nc.gpsimd.collective_compute(
                "AllGather", mybir.AluOpType.bypass,
                replica_groups=[list(range(NCORES))],
                ins=[e_bounce.opt()], outs=[ag_out.opt()])
            # ag_out row (2c+si) = core c stream si. Flat e index j*128+p with
            # chunk j = 9c+jj (ES = 9*128): view [two, p, c, jj].
            ag4 = ag_out[:].rearrange("(c two) (jj p) -> two p c jj", two=2, p=128)
            ss, scale1, g_sb, ob8 = {}, {}, {}, {}
            outb_cc = const.tile([128, KC], F32, tag="outb_cc")
            nc.sync.dma_start(outb_cc, A32("outb"))
            for si, s in enumerate(("m", "c")):
                sst = const.tile([128, 48], F32, tag=f"ss_{s}")
                for cc in range(5):
                    nc.sync.dma_start(sst[:, cc * 9:(cc + 1) * 9],
                                      ag4[si, :, cc, :])
                nc.sync.dma_start(sst[:, 45:48], ag4[si, :, 5, 0:3])
                s1 = const.tile([128, KC], F32, tag=f"s1_{s}")
                nc.vector.tensor_scalar_add(s1, sst[:, 24:48], 1.0)
                ss[s], scale1[s] = sst, s1
                gt = const.tile([128, KC], F32, tag=f"gate_{s}")
                nc.sync.dma_start(gt[:, 0:6], ag4[si, :, 5, 3:9])
                for cc in (6, 7):
                    nc.sync.dma_start(gt[:, 6 + (cc - 6) * 9:6 + (cc - 5) * 9],
                                      ag4[si, :, cc, :])
                g_sb[s] = gt
                ot = const.tile([128, KC], F32, tag=f"ob8_{s}")
                nc.vector.tensor_mul(ot, gt, outb_cc)
                nc.vector.tensor_scalar_mul(ot, ot, 1.0 / NCORES)
                ob8[s] = ot

            rbounce = dram.tile([16, S], F32)
            rb_n = [0]

            def row_bcast(row_ap, dst_tile):
                i = rb_n[0] % 16
                rb_n[0] += 1
                nc.sync.dma_start(rbounce[i:i + 1, :], row_ap)
                nc.sync.dma_start(dst_tile, bcast(rbounce[i, :]))

            spq, spk, spv, spmlp = {}, {}, {}, {}
            with tc.tile_pool(name="nh", bufs=1) as nhp:
                for si, s in enumerate(("m", "c")):
                    # ---------- phase N: layernorm + ada scale/shift ------
                    nhT = nhp.tile([128, KC, S], BF16, tag="nhT")
                    with tc.tile_pool(name="ph_n", bufs=1) as pn:
                        sum_ps = [psum.tile([1, 512], F32, tag="mm", name="sum_ps") for _ in range(2)]
                        sq_ps = [psum.tile([1, 512], F32, tag="mm", name="sq_ps") for _ in range(2)]
                        # x chunks stay resident across the two LN passes —
                        # one HBM read of x per stream instead of two.
                        xarena = pn.tile([128, KC, S], BF16, tag="xarena")
                        for kk in range(KC):
                            xk = xarena[:, kk, :]
                            nc.sync.dma_start(xk, A16(f"xT_{s}", kk * 128))
                            sq = pn.tile([128, S], BF16, tag=f"xsq{kk % 2}")
                            nc.vector.tensor_mul(sq, xk, xk)
                            for st in range(2):
                                sl = slice(st * 512, (st + 1) * 512)
                                nc.tensor.matmul(sum_ps[st], ones, xk[:, sl],
                                                 start=(kk == 0), stop=(kk == KC - 1))
                                nc.tensor.matmul(sq_ps[st], ones, sq[:, sl],
                                                 start=(kk == 0), stop=(kk == KC - 1))
                        mu = pn.tile([1, S], F32, tag="mu")
                        msq = pn.tile([1, S], F32, tag="msq")
                        for st in range(2):
                            sl = slice(st * 512, (st + 1) * 512)
                            nc.scalar.activation(mu[:, sl], sum_ps[st], AF.Copy,
                                                 scale=1.0 / D)
                            nc.scalar.activation(msq[:, sl], sq_ps[st], AF.Copy,
                                                 scale=1.0 / D)
                        var = pn.tile([1, S], F32, tag="var")
                        nc.vector.tensor_mul(var, mu, mu)
                        nc.vector.tensor_sub(var, msq, var)
                        rstd = pn.tile([1, S], F32, tag="rstd")
                        nc.scalar.activation(rstd, var, AF.Sqrt, bias=epst[:1, :])
                        nc.vector.reciprocal(rstd, rstd)
                        nmr = pn.tile([1, S], F32, tag="nmr")
                        nc.vector.tensor_mul(nmr, mu, rstd)
                        nc.vector.tensor_scalar_mul(nmr, nmr, -1.0)
                        rstd_bc = pn.tile([128, S], F32, tag="rstd_bc")
                        nmr_bc = pn.tile([128, S], F32, tag="nmr_bc")
                        row_bcast(rstd, rstd_bc)
                        row_bcast(nmr, nmr_bc)
                        for kk in range(KC):
                            t1 = pn.tile([128, S], F32, tag=f"t1{kk % 2}")
                            nc.vector.tensor_mul(t1, xarena[:, kk, :], rstd_bc)
                            nc.vector.tensor_add(t1, t1, nmr_bc)
                            nc.scalar.activation(nhT[:, kk, :], t1, AF.Identity,
                                                 bias=ss[s][:, kk:kk + 1],
                                                 scale=scale1[s][:, kk:kk + 1])

                    # ---------- phase QKV ---------------------------------
                    with (
                        tc.tile_pool(name="ph_qkv1", bufs=1) as p1,
                        tc.tile_pool(name="ph_qkv2", bufs=2) as p2,
                        tc.tile_pool(name="ph_qkvw", bufs=3) as pw,
                    ):
                        for pj in ("q", "k"):
                            pps = [[psum.tile([128, 512], F32, tag="mm", name="pps")
                                    for _ in range(2)] for _ in range(HPC)]
                            for kk in range(KC):
                                wt = pw.tile([128, QO], BF16, tag="wqk")
                                nc.sync.dma_start(
                                    wt, A16(f"w{pj}T_{s}", kk * 128))
                                for o in range(HPC):
                                    for st in range(2):
                                        nc.tensor.matmul(
                                            pps[o][st], wt[:, o * 128:(o + 1) * 128],
                                            nhT[:, kk, st * 512:(st + 1) * 512],
                                            start=(kk == 0), stop=(kk == KC - 1))
                            spill = dram.tile([QO, S], BF16, tag=f"sp_{pj}_{s}")
                            (spq if pj == "q" else spk)[s] = spill
                            for o in range(HPC):
                                raw = p2.tile([128, S], F32, tag="raw")
                                for st in range(2):
                                    sl = slice(st * 512, (st + 1) * 512)
                                    nc.scalar.activation(
                                        raw[:, sl], pps[o][st], AF.Identity,
                                        bias=qkb[(pj, s)][:, o:o + 1])
                                sqh = p1.tile([128, S], BF16, tag="sqh")
                                nc.vector.tensor_mul(sqh, raw, raw)
                                rps = [psum.tile([1, 512], F32, tag="mm", name="rps")
                                       for _ in range(2)]
                                msr = p1.tile([1, S], F32, tag="msr")
                                for st in range(2):
                                    sl = slice(st * 512, (st + 1) * 512)
                                    nc.tensor.matmul(rps[st], ones, sqh[:, sl],
                                                     start=True, stop=True)
                                    nc.scalar.activation(msr[:, sl], rps[st],
                                                         AF.Copy, scale=1.0 / 128)
                                rsr = p1.tile([1, S], F32, tag="rsr")
                                nc.scalar.activation(rsr, msr, AF.Sqrt, bias=epst[:1, :])
                                nc.vector.reciprocal(rsr, rsr)
                                rs_bc = p1.tile([128, S], F32, tag="rs_bc")
                                row_bcast(rsr, rs_bc)
                                wq = p1.tile([128, S], BF16, tag="wq")
                                nc.scalar.activation(wq, raw, AF.Copy,
                                                     scale=rwt[(pj, s)])
                                rot_ps = [psum.tile([128, 512], F32, tag="mm", name="rot_ps")
                                          for _ in range(2)]
                                fin = p2.tile([128, S], F32, tag="fin")
                                for st in range(2):
                                    sl = slice(st * 512, (st + 1) * 512)
                                    nc.tensor.matmul(rot_ps[st], rotT, wq[:, sl],
                                                     start=True, stop=True)
                                    nc.vector.tensor_mul(fin[:, sl], rot_ps[st],
                                                         sinT[:, sl])
                                t2 = p1.tile([128, S], F32, tag="t2")
                                nc.vector.tensor_mul(t2, wq, cosT)
                                nc.vector.tensor_add(fin, fin, t2)
                                fin16 = p2.tile([128, S], BF16, tag="fin16")
                                nc.vector.tensor_mul(fin16, fin, rs_bc)
                                nc.sync.dma_start(spill[o * 128:(o + 1) * 128, :], fin16)

                        # v projection (natural layout [seq, 384])
                        vps = [psum.tile([128, QO], F32, tag="mm", name="vps") for _ in range(8)]
                        for kk in range(KC):
                            wt = pw.tile([128, QO], BF16, tag="wqk")
                            nc.sync.dma_start(
                                wt, A16(f"wvT_{s}", kk * 128))
                            for sc in range(8):
                                nc.tensor.matmul(
                                    vps[sc], nhT[:, kk, sc * 128:(sc + 1) * 128],
                                    wt, start=(kk == 0), stop=(kk == KC - 1))
                        vsp = dram.tile([S, QO], BF16, tag=f"sp_v_{s}")
                        spv[s] = vsp
                        for sc in range(8):
                            vt = p1.tile([128, QO], BF16, tag="vt")
                            nc.vector.tensor_add(vt, vps[sc], vbb[s])
                            nc.sync.dma_start(vsp[sc * 128:(sc + 1) * 128, :], vt)

                    # ---------- phase MLP ---------------------------------
                    # MLP activations stay resident in SBUF through OUT-PROJ
                    # (24 KB/partition per stream) — no DRAM spill round-trip.
                    msp = mres.tile([128, MLPC // 128, S], BF16, tag=f"hmlp_{s}")
                    spmlp[s] = msp
                    with (
                        tc.tile_pool(name="ph_mlpw", bufs=3) as mw,
                    ):
                        for ob in range(3):
                            mps = [[psum.tile([128, 512], F32, tag="mm", name="mps")
                                    for _ in range(2)] for _ in range(4)]
                            for kk in range(KC):
                                wt = mw.tile([128, 512], BF16, tag="wmlp")
                                nc.sync.dma_start(
                                    wt, A16("mlpT", kk * 128, 128, ob * 512, 512))
                                for o4 in range(4):
                                    for st in range(2):
                                        nc.tensor.matmul(
                                            mps[o4][st],
                                            wt[:, o4 * 128:(o4 + 1) * 128],
                                            nhT[:, kk, st * 512:(st + 1) * 512],
                                            start=(kk == 0), stop=(kk == KC - 1))
                            for o4 in range(4):
                                o = ob * 4 + o4
                                for st in range(2):
                                    sl = slice(st * 512, (st + 1) * 512)
                                    nc.scalar.activation(msp[:, o, sl], mps[o4][st],
                                                         AF.Gelu_apprx_tanh,
                                                         bias=mbt[:, o:o + 1])

            # ---------------- phase ATTN ----------------------------------
            with tc.tile_pool(name="attn_out", bufs=1) as ao:
              with (
                tc.tile_pool(name="attn_qkv", bufs=1) as aq,
                tc.tile_pool(name="attn_wk", bufs=3) as awk,
                tc.tile_pool(name="attn_w1", bufs=2) as aw1,
              ):
                qm = aq.tile([128, HPC, S], BF16, tag="qm")
                am = ao.tile([128, HPC, S], BF16, tag="am")
                ac = ao.tile([128, HPC, S], BF16, tag="ac")
                att_out = {"m": am, "c": ac}
                for h in range(HPC):
                    nc.sync.dma_start(qm[:, h, :], spq["m"][h * 128:(h + 1) * 128, :])
                kt = vt = None
                for attn, (qs, ks, acc_t, fresh) in (
                    ("main", ("m", "m", am, True)),
                    ("ctrl", ("c", "c", ac, True)),
                    ("cross", ("m", "c", am, False)),
                ):
                    if attn != "cross":
                        kt = aq.tile([128, HPC, S], BF16, tag="kt")
                        vt = aq.tile([128, 8, QO], BF16, tag="vt")
                        for h in range(HPC):
                            nc.sync.dma_start(kt[:, h, :],
                                              spk[ks][h * 128:(h + 1) * 128, :])
                        for sc in range(8):
                            nc.sync.dma_start(vt[:, sc, :],
                                              spv[ks][sc * 128:(sc + 1) * 128, :])
                    if attn == "ctrl":
                        qt = aq.tile([128, HPC, S], BF16, tag="qc")
                        for h in range(HPC):
                            nc.sync.dma_start(qt[:, h, :],
                                              spq["c"][h * 128:(h + 1) * 128, :])
                    else:
                        qt = qm
                    for h in range(HPC):
                        av_ps = [psum.tile([128, 512], F32, tag="mm", name="av_ps") for _ in range(2)]
                        d_ps = [psum.tile([1, 512], F32, tag="mm", name="d_ps") for _ in range(2)]
                        for kv in range(8):
                            s_ps = [psum.tile([128, 512], F32, tag="mm", name="s_ps")
                                    for _ in range(2)]
                            et = awk.tile([128, S], BF16, tag="exp")
                            for st in range(2):
                                sl = slice(st * 512, (st + 1) * 512)
                                nc.tensor.matmul(s_ps[st],
                                                 kt[:, h, kv * 128:(kv + 1) * 128],
                                                 qt[:, h, sl], start=True, stop=True)
                                nc.scalar.activation(et[:, sl], s_ps[st], AF.Exp,
                                                     scale=INV_SQRT_HD)
                                nc.tensor.matmul(av_ps[st],
                                                 vt[:, kv, h * 128:(h + 1) * 128],
                                                 et[:, sl], start=(kv == 0),
                                                 stop=(kv == 7))
                                nc.tensor.matmul(d_ps[st], ones, et[:, sl],
                                                 start=(kv == 0), stop=(kv == 7))
                        den = aw1.tile([1, S], F32, tag="den")
                        for st in range(2):
                            nc.vector.reciprocal(den[:, st * 512:(st + 1) * 512],
                                                 d_ps[st])
                        den_bc = aw1.tile([128, S], F32, tag="den_bc")
                        row_bcast(den, den_bc)
                        for st in range(2):
                            sl = slice(st * 512, (st + 1) * 512)
                            if fresh:
                                nc.vector.tensor_mul(acc_t[:, h, sl], av_ps[st],
                                                     den_bc[:, sl])
                            else:
                                cr = aw1.tile([128, 512], F32, tag="crs")
                                nc.vector.tensor_mul(cr, av_ps[st], den_bc[:, sl])
                                nc.vector.tensor_add(acc_t[:, h, sl],
                                                     acc_t[:, h, sl], cr)

              # ---------------- phase OUT-PROJ --------------------------
              with (
                  tc.tile_pool(name="ow", bufs=2) as ow,
                  tc.tile_pool(name="fin", bufs=4) as fp,
              ):
                  mlp_sb = spmlp  # resident SBUF arenas from phase MLP
                  WOUT = W16["outT"]
                  for oc in range(KC):
                      warena = ow.tile([128, HKC, 128], BF16, tag="warena")
                      # one strided DMA for all 15 [128,128] weight chunks
                      nc.sync.dma_start(
                          warena,
                          bass.AP(tensor=b16.tensor,
                                  offset=b16.offset + OFF16["outT"] + oc * 128,
                                  ap=[[WOUT, 128], [128 * WOUT, HKC], [1, 128]]))
                      for s in ("m", "c"):
                          ops_t = [psum.tile([128, 512], F32, tag="mm", name="ops_t")
                                   for _ in range(2)]
                          for kk in range(HKC):
                              rh = (att_out[s][:, kk, :] if kk < HPC
                                    else mlp_sb[s][:, kk - HPC, :])
                              for st in range(2):
                                  nc.tensor.matmul(
                                      ops_t[st], warena[:, kk, :],
                                      rh[:, st * 512:(st + 1) * 512],
                                      start=(kk == 0), stop=(kk == HKC - 1))
                          tg = fp.tile([128, S], BF16, tag="tg")
                          xs = fp.tile([128, S], BF16, tag="xs")
                          nc.sync.dma_start(xs, A16(f"xT_{s}", oc * 128, 128))
                          xsf = fp.tile([128, S], F32, tag="xsf")
                          nc.vector.tensor_scalar_mul(xsf, xs, 1.0 / NCORES)
                          for st in range(2):
                              sl = slice(st * 512, (st + 1) * 512)
                              nc.scalar.activation(tg[:, sl], ops_t[st],
                                                   AF.Identity,
                                                   scale=g_sb[s][:, oc:oc + 1],
                                                   bias=ob8[s][:, oc:oc + 1])
                              nc.vector.tensor_add(tg[:, sl], tg[:, sl],
                                                   xsf[:, sl])
                          nc.sync.dma_start(
                              OUTMC[OUT_OFF[s] + oc * 128:
                                    OUT_OFF[s] + (oc + 1) * 128, :], tg)
    nc.compile()
    return nc


_NC_CACHE = []


def _get_nc():
    if not _NC_CACHE:
        _NC_CACHE.append(build_nc())
    return _NC_CACHE[0]


def _prep_core_inputs(inputs, c):
    f = np.float32
    bf = mybir.dt.np(BF16)
    T = lambda a: np.ascontiguousarray(np.asarray(a, f).T)
    TB = lambda a: np.ascontiguousarray(np.asarray(a, f).T.astype(bf))
    C = lambda a: np.ascontiguousarray(np.asarray(a, f))
    hs = np.asarray(inputs["hidden_states"], f)[0]
    hc = np.asarray(inputs["hidden_states_control"], f)[0]
    m = {}
    for s, x, te, nw, nb in (
        ("m", hs, inputs["temb"], inputs["norm_w"], inputs["norm_b"]),
        ("c", hc, inputs["temb_control"], inputs["normc_w"], inputs["normc_b"]),
    ):
        m[f"xT_{s}"] = TB(x)
        m[f"temb_{s}"] = C(np.asarray(te, f)[0].reshape(KC, 128).T)
        m[f"normT_{s}"] = TB(np.asarray(nw, f)[c * ES:(c + 1) * ES, :])
        m[f"nb_{s}"] = C(np.asarray(nb, f)[c * ES:(c + 1) * ES].reshape(1, ES))
    for s, pre in (("m", ""), ("c", "c")):
        for w in ("q", "k", "v"):
            W = np.asarray(inputs[f"{w}{pre}_w"], f)
            m[f"w{w}T_{s}"] = TB(W[c * QO:(c + 1) * QO, :])
        for w in ("q", "k"):
            b = np.asarray(inputs[f"{w}{pre}_b"], f)[c * QO:(c + 1) * QO]
            m[f"{w}b_{s}"] = C(b.reshape(HPC, 128).T)
        m[f"vb_{s}"] = C(np.asarray(inputs[f"v{pre}_b"], f)[c * QO:(c + 1) * QO]
                         .reshape(1, QO))
        m[f"rmsq_{s}"] = C(np.asarray(inputs["rms_q" + pre], f).reshape(128, 1))
        m[f"rmsk_{s}"] = C(np.asarray(inputs["rms_k" + pre], f).reshape(128, 1))
    m["mlpT"] = TB(np.asarray(inputs["mlp_w"], f)[c * MLPC:(c + 1) * MLPC, :])
    m["mlpb"] = C(np.asarray(inputs["mlp_b"], f)[c * MLPC:(c + 1) * MLPC]
                  .reshape(MLPC // 128, 128).T)
    ow = np.asarray(inputs["out_w"], f)
    m["outT"] = C(np.concatenate(
        [ow[:, c * QO:(c + 1) * QO], ow[:, D + c * MLPC:D + (c + 1) * MLPC]],
        axis=1).T).astype(bf)
    m["outb"] = C(np.asarray(inputs["out_b"], f).reshape(KC, 128).T)
    m["cosT"] = TB(inputs["rope_cos"])
    m["sinT"] = TB(inputs["rope_sin"])
    R = np.zeros((128, 128), f)
    for i in range(64):
        R[2 * i, 2 * i + 1] = -1.0
        R[2 * i + 1, 2 * i] = 1.0
    m["rotT"] = np.ascontiguousarray(R.T).astype(bf)
    for n, r, cc in BF16_SPECS:
        assert m[n].shape == (r, cc), (n, m[n].shape, (r, cc))
    for n, r, cc in F32_SPECS:
        assert m[n].shape == (r, cc), (n, m[n].shape, (r, cc))
    b16 = np.concatenate([np.asarray(m[n], bf).reshape(-1) for n, _, _ in BF16_SPECS])
    b32 = np.concatenate([np.asarray(m[n], f).reshape(-1) for n, _, _ in F32_SPECS])
    return {"b16": b16.reshape(1, N16), "b32": b32.reshape(1, N32)}


def run_cores(inputs, trace=False):
    nc = _get_nc()
    in_maps = [_prep_core_inputs(inputs, c) for c in range(NCORES)]
    res = run_bass_kernel_spmd(nc, in_maps, list(range(NCORES)), trace=trace)
    h = np.sum([np.asarray(r["out_mc"][:D], np.float32) for r in res.results],
               axis=0, dtype=np.float64)
    hc = np.sum([np.asarray(r["out_mc"][D:], np.float32) for r in res.results],
                axis=0, dtype=np.float64)
    h = np.ascontiguousarray(h.T.astype(np.float32)).reshape(1, S, D)
    hc = np.ascontiguousarray(hc.T.astype(np.float32)).reshape(1, S, D)
    return (h, hc), res


def kernel(**inputs):
    out, _ = run_cores(inputs, trace=False)
    return out

